# revision 5
# baseline (speedup 1.0000x reference)
"""Concept-attention (vq_codebook) Trainium2 kernel.

Reference computation (per batch n, with L = H*W spatial positions):
    theta  = w_theta @ x_n                  [FEAT, L]
    logits = theta.T @ pool                 [L, POOL]
    attn   = softmax(logits, axis=-1)
    agg    = pool @ attn.T                  [FEAT, L]
    o      = w_o @ agg                      [CH, L]
    out    = x + gamma * o

Sharding: data-parallel over (batch n, spatial half) -> 8 shards of
L=2048 pixel rows each; pool/weights replicated.

Per-core dataflow (all matmuls bf16 with fp32 PSUM accumulation):
    - layouts keep the pool axis p on SBUF partitions, so softmax
      normalization is deferred: we compute exp(logits) unnormalized
      (logits are bounded ~ +-25 here so no max-subtraction is needed),
      aggregate agg_unnorm = pool @ exp, and obtain the softmax
      denominator s[l] = sum_p exp via a cheap ones-vector matmul over a
      DVE-accumulated partial-sum tile.
    - The output projection is computed transposed, oT[l, c], so the
      1/s[l] normalization and the residual add are native per-partition
      DVE ops. gamma is folded into w_o on the host.

Host-side prep (cheap numpy): slices x per core, transposes weights/pool
to the layouts the matmuls want, casts to bf16, and transposes the
gathered per-core outputs back into [N, CH, H, W] float32.
"""

from contextlib import ExitStack

import numpy as np
import ml_dtypes

import concourse.tile as tile
from concourse import bacc, mybir
from concourse.bass_utils import run_bass_kernel_spmd

N, CH, H, W = 4, 512, 64, 64
HW = H * W                  # 4096
FEAT, POOL = 256, 8192
NCORES = 8
L = N * HW // NCORES        # 2048 pixel rows per core
NCHUNK = 4                  # l-chunks per core
LC = L // NCHUNK            # 512 = matmul moving size / PSUM bank
NPT = POOL // 128           # 64 pool partition-tiles
NSLOT = 8                   # DVE partial-sum accumulator slots

BF16 = mybir.dt.bfloat16
F32 = mybir.dt.float32

LAST_RESULTS = None         # BassKernelResults of the most recent run
_NC_CACHE = None


def _emit(nc, tc, ctx, xb, xt, wt, wo, pl, plt, out):
    singles = ctx.enter_context(tc.tile_pool(name="singles", bufs=1))
    th_pool = ctx.enter_context(tc.tile_pool(name="th", bufs=2))
    exp_pool = ctx.enter_context(tc.tile_pool(name="expp", bufs=3))
    sacc_pool = ctx.enter_context(tc.tile_pool(name="sacc", bufs=1))
    agg_pool = ctx.enter_context(tc.tile_pool(name="aggsb", bufs=2))
    small_pool = ctx.enter_context(tc.tile_pool(name="small", bufs=2))
    outp = ctx.enter_context(tc.tile_pool(name="outp", bufs=3))
    ps_lg = ctx.enter_context(tc.tile_pool(name="pslg", bufs=2, space="PSUM"))
    ps_agg = ctx.enter_context(tc.tile_pool(name="psagg", bufs=1, space="PSUM"))
    ps_misc = ctx.enter_context(tc.tile_pool(name="psmisc", bufs=2, space="PSUM"))

    # resident inputs
    xb_sb = singles.tile([128, CH // 128, L], BF16)
    nc.sync.dma_start(out=xb_sb, in_=xb.rearrange("(t p) l -> p t l", p=128))
    wt_sb = singles.tile([128, CH // 128, FEAT], BF16)
    nc.sync.dma_start(out=wt_sb, in_=wt.rearrange("(t p) f -> p t f", p=128))
    wo_sb = singles.tile([128, FEAT // 128, CH], BF16)
    nc.sync.dma_start(out=wo_sb, in_=wo.rearrange("(t p) c -> p t c", p=128))
    pl_sb = singles.tile([128, FEAT // 128, POOL], BF16)
    nc.sync.dma_start(out=pl_sb, in_=pl.rearrange("(t p) q -> p t q", p=128))
    plt_sb = singles.tile([128, NPT, FEAT], BF16)
    nc.sync.dma_start(out=plt_sb, in_=plt.rearrange("(t p) f -> p t f", p=128))
    xt_sb = singles.tile([128, L // 128, CH], F32)
    nc.sync.dma_start(out=xt_sb, in_=xt.rearrange("(j p) c -> p j c", p=128))
    ones_sb = singles.tile([128, 1], F32)
    nc.vector.memset(ones_sb, 1.0)

    for kc in range(NCHUNK):
        lsl = slice(kc * LC, (kc + 1) * LC)

        # theta[f, l] for this l-chunk, bf16 in SBUF
        theta_sb = th_pool.tile([128, 2, LC], BF16)
        for m in range(2):
            ps_th = ps_misc.tile([128, LC], F32, tag="misc")
            for ct in range(CH // 128):
                nc.tensor.matmul(
                    ps_th,
                    lhsT=wt_sb[:, ct, m * 128:(m + 1) * 128],
                    rhs=xb_sb[:, ct, lsl],
                    start=(ct == 0),
                    stop=(ct == CH // 128 - 1),
                )
            nc.scalar.copy(theta_sb[:, m, :], ps_th)

        # streaming pool attention: logits -> exp -> aggregate
        psum_agg = ps_agg.tile([128, 2, LC], F32)
        sacc = sacc_pool.tile([128, NSLOT, LC], BF16)
        for pp in range(NPT // 2):
            lg = ps_lg.tile([128, 2, LC], F32)
            for k in range(2):
                pt = 2 * pp + k
                psl = slice(pt * 128, (pt + 1) * 128)
                nc.tensor.matmul(
                    lg[:, k, :], lhsT=pl_sb[:, 0, psl], rhs=theta_sb[:, 0, :],
                    start=True, stop=False,
                )
                nc.tensor.matmul(
                    lg[:, k, :], lhsT=pl_sb[:, 1, psl], rhs=theta_sb[:, 1, :],
                    start=False, stop=True,
                )
            exp_t = exp_pool.tile([128, 2, LC], BF16)
            nc.scalar.activation(exp_t, lg, mybir.ActivationFunctionType.Exp)
            for k in range(2):
                pt = 2 * pp + k
                for m in range(2):
                    nc.tensor.matmul(
                        psum_agg[:, m, :],
                        lhsT=plt_sb[:, pt, m * 128:(m + 1) * 128],
                        rhs=exp_t[:, k, :],
                        start=(pt == 0),
                        stop=(pt == NPT - 1),
                    )
                slot = pt % NSLOT
                if pt < NSLOT:
                    nc.vector.tensor_copy(sacc[:, slot, :], exp_t[:, k, :])
                else:
                    nc.vector.tensor_add(
                        sacc[:, slot, :], sacc[:, slot, :], exp_t[:, k, :]
                    )

        # softmax denominator s[l]: fold 8 slots, then ones.T @ sacc
        s01 = small_pool.tile([128, 4, LC], F32, tag="sfold")
        for q in range(4):
            nc.vector.tensor_add(
                s01[:, q, :], sacc[:, 2 * q, :], sacc[:, 2 * q + 1, :]
            )
        nc.vector.tensor_add(s01[:, 0, :], s01[:, 0, :], s01[:, 1, :])
        nc.vector.tensor_add(s01[:, 2, :], s01[:, 2, :], s01[:, 3, :])
        nc.vector.tensor_add(s01[:, 0, :], s01[:, 0, :], s01[:, 2, :])
        ps_s = ps_misc.tile([1, LC], F32, tag="misc")
        nc.tensor.matmul(ps_s, lhsT=ones_sb, rhs=s01[:, 0, :], start=True, stop=True)
        s_row = small_pool.tile([1, LC], F32, tag="srow")
        nc.vector.tensor_copy(s_row, ps_s)

        # transpose s row -> per-partition column layout, reciprocal
        scol = small_pool.tile([128, LC // 128], F32, tag="scol")
        for j in range(LC // 128):
            nc.sync.dma_start(
                out=scol[:, j:j + 1], in_=s_row[0:1, j * 128:(j + 1) * 128]
            )
        rcol = small_pool.tile([128, LC // 128], F32, tag="rcol")
        nc.vector.reciprocal(rcol, scol)

        # agg psum -> sbuf bf16 (lhsT of the output projection)
        agg_sb = agg_pool.tile([128, 2, LC], BF16)
        for m in range(2):
            nc.scalar.copy(agg_sb[:, m, :], psum_agg[:, m, :])

        # oT[l, c] = agg.T @ (gamma*w_o.T); normalize by 1/s[l]; add x.T
        for j in range(LC // 128):
            jl = kc * (LC // 128) + j
            ps_o = ps_misc.tile([128, CH], F32, tag="misc")
            for m in range(2):
                nc.tensor.matmul(
                    ps_o,
                    lhsT=agg_sb[:, m, j * 128:(j + 1) * 128],
                    rhs=wo_sb[:, m, :],
                    start=(m == 0),
                    stop=(m == 1),
                )
            out_sb = outp.tile([128, CH], F32)
            nc.vector.scalar_tensor_tensor(
                out=out_sb,
                in0=ps_o,
                scalar=rcol[:, j:j + 1],
                in1=xt_sb[:, jl, :],
                op0=mybir.AluOpType.mult,
                op1=mybir.AluOpType.add,
            )
            nc.sync.dma_start(out=out[jl * 128:(jl + 1) * 128, :], in_=out_sb)


def _build_nc():
    nc = bacc.Bacc("TRN2", target_bir_lowering=False, debug=False)
    xb = nc.dram_tensor("xb", [CH, L], BF16, kind="ExternalInput")
    xt = nc.dram_tensor("xt", [L, CH], F32, kind="ExternalInput")
    wt = nc.dram_tensor("wt", [CH, FEAT], BF16, kind="ExternalInput")
    wo = nc.dram_tensor("wo", [FEAT, CH], BF16, kind="ExternalInput")
    pl = nc.dram_tensor("pl", [FEAT, POOL], BF16, kind="ExternalInput")
    plt = nc.dram_tensor("plt", [POOL, FEAT], BF16, kind="ExternalInput")
    out = nc.dram_tensor("out", [L, CH], F32, kind="ExternalOutput")
    with tile.TileContext(nc) as tc, ExitStack() as ctx:
        _emit(nc, tc, ctx, xb[:], xt[:], wt[:], wo[:], pl[:], plt[:], out[:])
    nc.compile()
    return nc


def kernel(x, w_theta, w_o, concept_pool, gamma):
    global _NC_CACHE, LAST_RESULTS
    if _NC_CACHE is None:
        _NC_CACHE = _build_nc()
    nc = _NC_CACHE

    bf = ml_dtypes.bfloat16
    x = np.asarray(x, dtype=np.float32).reshape(N, CH, HW)
    wt_h = np.ascontiguousarray(np.asarray(w_theta, np.float32).T).astype(bf)
    wo_h = np.ascontiguousarray(
        (np.float32(gamma) * np.asarray(w_o, np.float32)).T
    ).astype(bf)
    pl_h = np.ascontiguousarray(np.asarray(concept_pool, np.float32)).astype(bf)
    plt_h = np.ascontiguousarray(np.asarray(concept_pool, np.float32).T).astype(bf)

    in_maps = []
    for c in range(NCORES):
        n, half = divmod(c, NCORES // N)
        xc = x[n][:, half * L:(half + 1) * L]
        in_maps.append({
            "xb": np.ascontiguousarray(xc).astype(bf),
            "xt": np.ascontiguousarray(xc.T),
            "wt": wt_h,
            "wo": wo_h,
            "pl": pl_h,
            "plt": plt_h,
        })

    LAST_RESULTS = run_bass_kernel_spmd(nc, in_maps, list(range(NCORES)))

    out = np.empty((N, CH, HW), dtype=np.float32)
    for c in range(NCORES):
        n, half = divmod(c, NCORES // N)
        out[n][:, half * L:(half + 1) * L] = LAST_RESULTS.results[c]["out"].T
    return out.reshape(N, CH, H, W)


# revision 11
# speedup vs baseline: 1.1577x; 1.1577x over previous
"""Concept-attention (vq_codebook) Trainium2 kernel.

Reference computation (per batch n, with L = H*W spatial positions):
    theta  = w_theta @ x_n                  [FEAT, L]
    logits = theta.T @ pool                 [L, POOL]
    attn   = softmax(logits, axis=-1)
    agg    = pool @ attn.T                  [FEAT, L]
    o      = w_o @ agg                      [CH, L]
    out    = x + gamma * o

Sharding: data-parallel over (batch n, spatial half) -> 8 shards of
L=2048 pixel rows each; pool/weights replicated.

Per-core dataflow (all matmuls bf16 with fp32 PSUM accumulation):
    - layouts keep the pool axis p on SBUF partitions, so softmax
      normalization is deferred: we compute exp(logits) unnormalized
      (logits are bounded ~ +-25 here so no max-subtraction is needed),
      aggregate agg_unnorm = pool @ exp, and obtain the softmax
      denominator s[l] = sum_p exp via a cheap ones-vector matmul over a
      DVE-accumulated partial-sum tile.
    - The output projection is computed transposed, oT[l, c], so the
      1/s[l] normalization and the residual add are native per-partition
      DVE ops. gamma is folded into w_o on the host.

Host-side prep (cheap numpy): slices x per core, transposes weights/pool
to the layouts the matmuls want, casts to bf16, and transposes the
gathered per-core outputs back into [N, CH, H, W] float32.
"""

from contextlib import ExitStack

import numpy as np
import ml_dtypes

import concourse.tile as tile
from concourse import bacc, mybir
from concourse.bass_utils import run_bass_kernel_spmd

N, CH, H, W = 4, 512, 64, 64
HW = H * W                  # 4096
FEAT, POOL = 256, 8192
NCORES = 8
L = N * HW // NCORES        # 2048 pixel rows per core
NCHUNK = 4                  # l-chunks per core
LC = L // NCHUNK            # 512 = matmul moving size / PSUM bank
NPT = POOL // 128           # 64 pool partition-tiles
NSLOT = 8                   # DVE partial-sum accumulator slots

BF16 = mybir.dt.bfloat16
F32 = mybir.dt.float32

LAST_RESULTS = None         # BassKernelResults of the most recent run
_NC_CACHE = None


def _emit(nc, tc, ctx, xb, xt, wt, wo, pl, plt, out):
    singles = ctx.enter_context(tc.tile_pool(name="singles", bufs=1))
    th_pool = ctx.enter_context(tc.tile_pool(name="th", bufs=2))
    exp_pool = ctx.enter_context(tc.tile_pool(name="expp", bufs=3))
    sacc_pool = ctx.enter_context(tc.tile_pool(name="sacc", bufs=1))
    agg_pool = ctx.enter_context(tc.tile_pool(name="aggsb", bufs=2))
    small_pool = ctx.enter_context(tc.tile_pool(name="small", bufs=2))
    outp = ctx.enter_context(tc.tile_pool(name="outp", bufs=3))
    ps_lg = ctx.enter_context(tc.tile_pool(name="pslg", bufs=2, space="PSUM"))
    ps_agg = ctx.enter_context(tc.tile_pool(name="psagg", bufs=1, space="PSUM"))
    ps_th = ctx.enter_context(tc.tile_pool(name="psth", bufs=1, space="PSUM"))
    ps_misc = ctx.enter_context(tc.tile_pool(name="psmisc", bufs=1, space="PSUM"))

    # resident inputs; spread across engine DMA queues and chunked in
    # consumption order so compute starts as soon as the first slices land
    wt_sb = singles.tile([128, CH // 128, FEAT], BF16)
    nc.sync.dma_start(out=wt_sb, in_=wt.rearrange("(t p) f -> p t f", p=128))
    xb_sb = singles.tile([128, CH // 128, L], BF16)
    nc.sync.dma_start(out=xb_sb, in_=xb.rearrange("(t p) l -> p t l", p=128))
    wo_sb = singles.tile([128, FEAT // 128, CH], BF16)
    nc.scalar.dma_start(out=wo_sb, in_=wo.rearrange("(t p) c -> p t c", p=128))
    NSPL = 4
    PSPL = POOL // NSPL
    pl_sbs, plt_sbs = [], []
    pl_r = pl.rearrange("(t p) q -> p t q", p=128)
    plt_r = plt.rearrange("(t p) f -> p t f", p=128)
    for i in range(NSPL):
        pl_i = singles.tile([128, FEAT // 128, PSPL], BF16, name=f"pl_{i}")
        nc.gpsimd.dma_start(out=pl_i, in_=pl_r[:, :, i * PSPL:(i + 1) * PSPL])
        pl_sbs.append(pl_i)
        plt_i = singles.tile([128, NPT // NSPL, FEAT], BF16, name=f"plt_{i}")
        nc.gpsimd.dma_start(
            out=plt_i, in_=plt_r[:, i * (NPT // NSPL):(i + 1) * (NPT // NSPL), :]
        )
        plt_sbs.append(plt_i)
    xt_sbs = []
    xt_r = xt.rearrange("(j p) c -> p j c", p=128)
    JC = LC // 128
    for kc in range(NCHUNK):
        xt_i = singles.tile([128, JC, CH], F32, name=f"xt_{kc}")
        nc.scalar.dma_start(out=xt_i, in_=xt_r[:, kc * JC:(kc + 1) * JC, :])
        xt_sbs.append(xt_i)
    ones_sb = singles.tile([128, 1], F32)
    nc.vector.memset(ones_sb, 1.0)

    def pl_slice(ft, pt):
        i, r = divmod(pt, NPT // NSPL)
        return pl_sbs[i][:, ft, r * 128:(r + 1) * 128]

    def plt_slice(pt, m):
        i, r = divmod(pt, NPT // NSPL)
        return plt_sbs[i][:, r, m * 128:(m + 1) * 128]

    for kc in range(NCHUNK):
        lsl = slice(kc * LC, (kc + 1) * LC)

        # theta[f, l] for this l-chunk, bf16 in SBUF
        theta_sb = th_pool.tile([128, 2, LC], BF16)
        for m in range(2):
            ps_t = ps_th.tile([128, LC], F32, tag="th")
            for ct in range(CH // 128):
                nc.tensor.matmul(
                    ps_t,
                    lhsT=wt_sb[:, ct, m * 128:(m + 1) * 128],
                    rhs=xb_sb[:, ct, lsl],
                    start=(ct == 0),
                    stop=(ct == CH // 128 - 1),
                )
            nc.scalar.copy(theta_sb[:, m, :], ps_t)

        # streaming pool attention: logits -> exp -> aggregate
        psum_agg = ps_agg.tile([128, 2, LC], F32)
        sacc = sacc_pool.tile([128, NSLOT, LC], BF16)
        for pp in range(NPT // 2):
            lg = ps_lg.tile([128, 2, LC], F32)
            for k in range(2):
                pt = 2 * pp + k
                nc.tensor.matmul(
                    lg[:, k, :], lhsT=pl_slice(0, pt), rhs=theta_sb[:, 0, :],
                    start=True, stop=False,
                )
                nc.tensor.matmul(
                    lg[:, k, :], lhsT=pl_slice(1, pt), rhs=theta_sb[:, 1, :],
                    start=False, stop=True,
                )
            exp_t = exp_pool.tile([128, 2, LC], BF16)
            nc.scalar.activation(exp_t, lg, mybir.ActivationFunctionType.Exp)
            for k in range(2):
                pt = 2 * pp + k
                for m in range(2):
                    nc.tensor.matmul(
                        psum_agg[:, m, :],
                        lhsT=plt_slice(pt, m),
                        rhs=exp_t[:, k, :],
                        start=(pt == 0),
                        stop=(pt == NPT - 1),
                    )
                slot = pt % NSLOT
                if pt < NSLOT:
                    nc.vector.tensor_copy(sacc[:, slot, :], exp_t[:, k, :])
                else:
                    nc.vector.tensor_add(
                        sacc[:, slot, :], sacc[:, slot, :], exp_t[:, k, :]
                    )

        # softmax denominator s[l]: fold 8 slots, then ones.T @ sacc
        s01 = small_pool.tile([128, 4, LC], F32, tag="sfold")
        for q in range(4):
            nc.vector.tensor_add(
                s01[:, q, :], sacc[:, 2 * q, :], sacc[:, 2 * q + 1, :]
            )
        nc.vector.tensor_add(s01[:, 0, :], s01[:, 0, :], s01[:, 1, :])
        nc.vector.tensor_add(s01[:, 2, :], s01[:, 2, :], s01[:, 3, :])
        nc.vector.tensor_add(s01[:, 0, :], s01[:, 0, :], s01[:, 2, :])
        ps_s = ps_misc.tile([1, LC], F32, tag="misc")
        nc.tensor.matmul(ps_s, lhsT=ones_sb, rhs=s01[:, 0, :], start=True, stop=True)
        s_row = small_pool.tile([1, LC], F32, tag="srow")
        nc.vector.tensor_copy(s_row, ps_s)

        # transpose s row -> per-partition column layout, reciprocal
        scol = small_pool.tile([128, LC // 128], F32, tag="scol")
        for j in range(LC // 128):
            nc.sync.dma_start(
                out=scol[:, j:j + 1], in_=s_row[0:1, j * 128:(j + 1) * 128]
            )
        rcol = small_pool.tile([128, LC // 128], F32, tag="rcol")
        nc.vector.reciprocal(rcol, scol)

        # agg psum -> sbuf bf16 (lhsT of the output projection)
        agg_sb = agg_pool.tile([128, 2, LC], BF16)
        for m in range(2):
            nc.scalar.copy(agg_sb[:, m, :], psum_agg[:, m, :])

        # oT[l, c] = agg.T @ (gamma*w_o.T); normalize by 1/s[l]; add x.T
        for j in range(LC // 128):
            jl = kc * (LC // 128) + j
            ps_o = ps_misc.tile([128, CH], F32, tag="misc")
            for m in range(2):
                nc.tensor.matmul(
                    ps_o,
                    lhsT=agg_sb[:, m, j * 128:(j + 1) * 128],
                    rhs=wo_sb[:, m, :],
                    start=(m == 0),
                    stop=(m == 1),
                )
            out_sb = outp.tile([128, CH], F32)
            nc.vector.scalar_tensor_tensor(
                out=out_sb,
                in0=ps_o,
                scalar=rcol[:, j:j + 1],
                in1=xt_sbs[kc][:, j, :],
                op0=mybir.AluOpType.mult,
                op1=mybir.AluOpType.add,
            )
            nc.sync.dma_start(out=out[jl * 128:(jl + 1) * 128, :], in_=out_sb)


def _build_nc():
    nc = bacc.Bacc("TRN2", target_bir_lowering=False, debug=False)
    xb = nc.dram_tensor("xb", [CH, L], BF16, kind="ExternalInput")
    xt = nc.dram_tensor("xt", [L, CH], F32, kind="ExternalInput")
    wt = nc.dram_tensor("wt", [CH, FEAT], BF16, kind="ExternalInput")
    wo = nc.dram_tensor("wo", [FEAT, CH], BF16, kind="ExternalInput")
    pl = nc.dram_tensor("pl", [FEAT, POOL], BF16, kind="ExternalInput")
    plt = nc.dram_tensor("plt", [POOL, FEAT], BF16, kind="ExternalInput")
    out = nc.dram_tensor("out", [L, CH], F32, kind="ExternalOutput")
    with tile.TileContext(nc) as tc, ExitStack() as ctx:
        _emit(nc, tc, ctx, xb[:], xt[:], wt[:], wo[:], pl[:], plt[:], out[:])
    nc.compile()
    return nc


def kernel(x, w_theta, w_o, concept_pool, gamma):
    global _NC_CACHE, LAST_RESULTS
    if _NC_CACHE is None:
        _NC_CACHE = _build_nc()
    nc = _NC_CACHE

    bf = ml_dtypes.bfloat16
    x = np.asarray(x, dtype=np.float32).reshape(N, CH, HW)
    wt_h = np.ascontiguousarray(np.asarray(w_theta, np.float32).T).astype(bf)
    wo_h = np.ascontiguousarray(
        (np.float32(gamma) * np.asarray(w_o, np.float32)).T
    ).astype(bf)
    pl_h = np.ascontiguousarray(np.asarray(concept_pool, np.float32)).astype(bf)
    plt_h = np.ascontiguousarray(np.asarray(concept_pool, np.float32).T).astype(bf)

    in_maps = []
    for c in range(NCORES):
        n, half = divmod(c, NCORES // N)
        xc = x[n][:, half * L:(half + 1) * L]
        in_maps.append({
            "xb": np.ascontiguousarray(xc).astype(bf),
            "xt": np.ascontiguousarray(xc.T),
            "wt": wt_h,
            "wo": wo_h,
            "pl": pl_h,
            "plt": plt_h,
        })

    LAST_RESULTS = run_bass_kernel_spmd(nc, in_maps, list(range(NCORES)))

    out = np.empty((N, CH, HW), dtype=np.float32)
    for c in range(NCORES):
        n, half = divmod(c, NCORES // N)
        out[n][:, half * L:(half + 1) * L] = LAST_RESULTS.results[c]["out"].T
    return out.reshape(N, CH, H, W)


# revision 13
# speedup vs baseline: 1.1617x; 1.0035x over previous
"""Concept-attention (vq_codebook) Trainium2 kernel.

Reference computation (per batch n, with L = H*W spatial positions):
    theta  = w_theta @ x_n                  [FEAT, L]
    logits = theta.T @ pool                 [L, POOL]
    attn   = softmax(logits, axis=-1)
    agg    = pool @ attn.T                  [FEAT, L]
    o      = w_o @ agg                      [CH, L]
    out    = x + gamma * o

Sharding: data-parallel over (batch n, spatial half) -> 8 shards of
L=2048 pixel rows each; pool/weights replicated.

Per-core dataflow (all matmuls bf16 with fp32 PSUM accumulation):
    - layouts keep the pool axis p on SBUF partitions, so softmax
      normalization is deferred: we compute exp(logits) unnormalized
      (logits are bounded ~ +-25 here so no max-subtraction is needed),
      aggregate agg_unnorm = pool @ exp, and obtain the softmax
      denominator s[l] = sum_p exp via a cheap ones-vector matmul over a
      DVE-accumulated partial-sum tile.
    - The output projection is computed transposed, oT[l, c], so the
      1/s[l] normalization and the residual add are native per-partition
      DVE ops. gamma is folded into w_o on the host.

Host-side prep (cheap numpy): slices x per core, transposes weights/pool
to the layouts the matmuls want, casts to bf16, and transposes the
gathered per-core outputs back into [N, CH, H, W] float32.
"""

from contextlib import ExitStack

import numpy as np
import ml_dtypes

import concourse.tile as tile
from concourse import bacc, mybir
from concourse.bass_utils import run_bass_kernel_spmd

N, CH, H, W = 4, 512, 64, 64
HW = H * W                  # 4096
FEAT, POOL = 256, 8192
NCORES = 8
L = N * HW // NCORES        # 2048 pixel rows per core
NCHUNK = 4                  # l-chunks per core
LC = L // NCHUNK            # 512 = matmul moving size / PSUM bank
NPT = POOL // 128           # 64 pool partition-tiles
NSLOT = 8                   # DVE partial-sum accumulator slots

BF16 = mybir.dt.bfloat16
F32 = mybir.dt.float32

LAST_RESULTS = None         # BassKernelResults of the most recent run
_NC_CACHE = None


def _emit(nc, tc, ctx, xb, xt, wt, wo, pl, plt, out):
    singles = ctx.enter_context(tc.tile_pool(name="singles", bufs=1))
    th_pool = ctx.enter_context(tc.tile_pool(name="th", bufs=2))
    exp_pool = ctx.enter_context(tc.tile_pool(name="expp", bufs=3))
    sacc_pool = ctx.enter_context(tc.tile_pool(name="sacc", bufs=1))
    agg_pool = ctx.enter_context(tc.tile_pool(name="aggsb", bufs=2))
    small_pool = ctx.enter_context(tc.tile_pool(name="small", bufs=2))
    outp = ctx.enter_context(tc.tile_pool(name="outp", bufs=3))
    ps_lg = ctx.enter_context(tc.tile_pool(name="pslg", bufs=2, space="PSUM"))
    ps_agg = ctx.enter_context(tc.tile_pool(name="psagg", bufs=1, space="PSUM"))
    ps_th = ctx.enter_context(tc.tile_pool(name="psth", bufs=1, space="PSUM"))
    ps_misc = ctx.enter_context(tc.tile_pool(name="psmisc", bufs=1, space="PSUM"))

    # resident inputs; spread across engine DMA queues, chunked finely and
    # emitted in consumption order so compute starts as soon as the first
    # slices land. xt (residual, needed only at chunk tails) is loaded from
    # inside the chunk loop so it doesn't compete with the startup loads.
    wt_sb = singles.tile([128, CH // 128, FEAT], BF16)
    nc.sync.dma_start(out=wt_sb, in_=wt.rearrange("(t p) f -> p t f", p=128))
    xb_r = xb.rearrange("(t p) (kc l) -> p t kc l", p=128, kc=NCHUNK)
    xb_sbs = []
    for kc in range(NCHUNK):
        xb_i = singles.tile([128, CH // 128, LC], BF16, name=f"xb_{kc}")
        nc.sync.dma_start(out=xb_i, in_=xb_r[:, :, kc, :])
        xb_sbs.append(xb_i)
    NSPL = 8
    PSPL = POOL // NSPL
    pl_sbs, plt_sbs = [], []
    pl_r = pl.rearrange("(t p) q -> p t q", p=128)
    plt_r = plt.rearrange("(t p) f -> p t f", p=128)
    for i in range(NSPL):
        pl_i = singles.tile([128, FEAT // 128, PSPL], BF16, name=f"pl_{i}")
        nc.gpsimd.dma_start(out=pl_i, in_=pl_r[:, :, i * PSPL:(i + 1) * PSPL])
        pl_sbs.append(pl_i)
        plt_i = singles.tile([128, NPT // NSPL, FEAT], BF16, name=f"plt_{i}")
        nc.gpsimd.dma_start(
            out=plt_i, in_=plt_r[:, i * (NPT // NSPL):(i + 1) * (NPT // NSPL), :]
        )
        plt_sbs.append(plt_i)
    wo_sb = singles.tile([128, FEAT // 128, CH], BF16)
    nc.scalar.dma_start(out=wo_sb, in_=wo.rearrange("(t p) c -> p t c", p=128))
    xt_r = xt.rearrange("(j p) c -> p j c", p=128)
    JC = LC // 128
    xt_sbs = [
        singles.tile([128, JC, CH], F32, name=f"xt_{kc}") for kc in range(NCHUNK)
    ]
    ones_sb = singles.tile([128, 1], F32)
    nc.vector.memset(ones_sb, 1.0)

    def pl_slice(ft, pt):
        i, r = divmod(pt, NPT // NSPL)
        return pl_sbs[i][:, ft, r * 128:(r + 1) * 128]

    def plt_slice(pt, m):
        i, r = divmod(pt, NPT // NSPL)
        return plt_sbs[i][:, r, m * 128:(m + 1) * 128]

    for kc in range(NCHUNK):
        # residual input for this chunk (needed only at the chunk tail)
        nc.scalar.dma_start(
            out=xt_sbs[kc], in_=xt_r[:, kc * JC:(kc + 1) * JC, :]
        )

        # theta[f, l] for this l-chunk, bf16 in SBUF
        theta_sb = th_pool.tile([128, 2, LC], BF16)
        for m in range(2):
            ps_t = ps_th.tile([128, LC], F32, tag="th")
            for ct in range(CH // 128):
                nc.tensor.matmul(
                    ps_t,
                    lhsT=wt_sb[:, ct, m * 128:(m + 1) * 128],
                    rhs=xb_sbs[kc][:, ct, :],
                    start=(ct == 0),
                    stop=(ct == CH // 128 - 1),
                )
            nc.scalar.copy(theta_sb[:, m, :], ps_t)

        # streaming pool attention: logits -> exp -> aggregate.
        # s-partials use slot = pt // 8 so slots complete early and the
        # fold tree overlaps the tail of the loop.
        psum_agg = ps_agg.tile([128, 2, LC], F32)
        sacc = sacc_pool.tile([128, NSLOT, LC], BF16)
        sf = small_pool.tile([128, 7, LC], F32, tag="sfold")
        PERSLOT = NPT // NSLOT
        for pp in range(NPT // 2):
            lg = ps_lg.tile([128, 2, LC], F32)
            for k in range(2):
                pt = 2 * pp + k
                nc.tensor.matmul(
                    lg[:, k, :], lhsT=pl_slice(0, pt), rhs=theta_sb[:, 0, :],
                    start=True, stop=False,
                )
                nc.tensor.matmul(
                    lg[:, k, :], lhsT=pl_slice(1, pt), rhs=theta_sb[:, 1, :],
                    start=False, stop=True,
                )
            exp_t = exp_pool.tile([128, 2, LC], BF16)
            nc.scalar.activation(exp_t, lg, mybir.ActivationFunctionType.Exp)
            for k in range(2):
                pt = 2 * pp + k
                for m in range(2):
                    nc.tensor.matmul(
                        psum_agg[:, m, :],
                        lhsT=plt_slice(pt, m),
                        rhs=exp_t[:, k, :],
                        start=(pt == 0),
                        stop=(pt == NPT - 1),
                    )
                slot, r = divmod(pt, PERSLOT)
                if r == 0:
                    nc.vector.tensor_copy(sacc[:, slot, :], exp_t[:, k, :])
                else:
                    nc.vector.tensor_add(
                        sacc[:, slot, :], sacc[:, slot, :], exp_t[:, k, :]
                    )
                # progressive fold: slot q completes once pt == (q+1)*PERSLOT-1
                if r == PERSLOT - 1 and slot % 2 == 1:
                    q = slot // 2  # sf[q] = sacc[2q] + sacc[2q+1]
                    nc.vector.tensor_add(
                        sf[:, q, :], sacc[:, 2 * q, :], sacc[:, 2 * q + 1, :]
                    )
                    if q % 2 == 1:
                        nc.vector.tensor_add(
                            sf[:, 4 + q // 2, :], sf[:, q - 1, :], sf[:, q, :]
                        )

        # softmax denominator s[l]: final fold, then ones.T @ s
        nc.vector.tensor_add(sf[:, 6, :], sf[:, 4, :], sf[:, 5, :])
        ps_s = ps_misc.tile([1, LC], F32, tag="misc")
        nc.tensor.matmul(ps_s, lhsT=ones_sb, rhs=sf[:, 6, :], start=True, stop=True)
        s_row = small_pool.tile([1, LC], F32, tag="srow")
        nc.vector.tensor_copy(s_row, ps_s)

        # transpose s row -> per-partition column layout, reciprocal
        scol = small_pool.tile([128, LC // 128], F32, tag="scol")
        for j in range(LC // 128):
            nc.sync.dma_start(
                out=scol[:, j:j + 1], in_=s_row[0:1, j * 128:(j + 1) * 128]
            )
        rcol = small_pool.tile([128, LC // 128], F32, tag="rcol")
        nc.vector.reciprocal(rcol, scol)

        # agg psum -> sbuf bf16 (lhsT of the output projection)
        agg_sb = agg_pool.tile([128, 2, LC], BF16)
        for m in range(2):
            nc.scalar.copy(agg_sb[:, m, :], psum_agg[:, m, :])

        # oT[l, c] = agg.T @ (gamma*w_o.T); normalize by 1/s[l]; add x.T
        for j in range(LC // 128):
            jl = kc * (LC // 128) + j
            ps_o = ps_misc.tile([128, CH], F32, tag="misc")
            for m in range(2):
                nc.tensor.matmul(
                    ps_o,
                    lhsT=agg_sb[:, m, j * 128:(j + 1) * 128],
                    rhs=wo_sb[:, m, :],
                    start=(m == 0),
                    stop=(m == 1),
                )
            out_sb = outp.tile([128, CH], F32)
            nc.vector.scalar_tensor_tensor(
                out=out_sb,
                in0=ps_o,
                scalar=rcol[:, j:j + 1],
                in1=xt_sbs[kc][:, j, :],
                op0=mybir.AluOpType.mult,
                op1=mybir.AluOpType.add,
            )
            nc.sync.dma_start(out=out[jl * 128:(jl + 1) * 128, :], in_=out_sb)


def _build_nc():
    nc = bacc.Bacc("TRN2", target_bir_lowering=False, debug=False)
    xb = nc.dram_tensor("xb", [CH, L], BF16, kind="ExternalInput")
    xt = nc.dram_tensor("xt", [L, CH], F32, kind="ExternalInput")
    wt = nc.dram_tensor("wt", [CH, FEAT], BF16, kind="ExternalInput")
    wo = nc.dram_tensor("wo", [FEAT, CH], BF16, kind="ExternalInput")
    pl = nc.dram_tensor("pl", [FEAT, POOL], BF16, kind="ExternalInput")
    plt = nc.dram_tensor("plt", [POOL, FEAT], BF16, kind="ExternalInput")
    out = nc.dram_tensor("out", [L, CH], F32, kind="ExternalOutput")
    with tile.TileContext(nc) as tc, ExitStack() as ctx:
        _emit(nc, tc, ctx, xb[:], xt[:], wt[:], wo[:], pl[:], plt[:], out[:])
    nc.compile()
    return nc


def kernel(x, w_theta, w_o, concept_pool, gamma):
    global _NC_CACHE, LAST_RESULTS
    if _NC_CACHE is None:
        _NC_CACHE = _build_nc()
    nc = _NC_CACHE

    bf = ml_dtypes.bfloat16
    x = np.asarray(x, dtype=np.float32).reshape(N, CH, HW)
    wt_h = np.ascontiguousarray(np.asarray(w_theta, np.float32).T).astype(bf)
    wo_h = np.ascontiguousarray(
        (np.float32(gamma) * np.asarray(w_o, np.float32)).T
    ).astype(bf)
    pl_h = np.ascontiguousarray(np.asarray(concept_pool, np.float32)).astype(bf)
    plt_h = np.ascontiguousarray(np.asarray(concept_pool, np.float32).T).astype(bf)

    in_maps = []
    for c in range(NCORES):
        n, half = divmod(c, NCORES // N)
        xc = x[n][:, half * L:(half + 1) * L]
        in_maps.append({
            "xb": np.ascontiguousarray(xc).astype(bf),
            "xt": np.ascontiguousarray(xc.T),
            "wt": wt_h,
            "wo": wo_h,
            "pl": pl_h,
            "plt": plt_h,
        })

    LAST_RESULTS = run_bass_kernel_spmd(nc, in_maps, list(range(NCORES)))

    out = np.empty((N, CH, HW), dtype=np.float32)
    for c in range(NCORES):
        n, half = divmod(c, NCORES // N)
        out[n][:, half * L:(half + 1) * L] = LAST_RESULTS.results[c]["out"].T
    return out.reshape(N, CH, H, W)


# revision 19
# speedup vs baseline: 1.2152x; 1.0460x over previous
"""Concept-attention (vq_codebook) Trainium2 kernel.

Reference computation (per batch n, with L = H*W spatial positions):
    theta  = w_theta @ x_n                  [FEAT, L]
    logits = theta.T @ pool                 [L, POOL]
    attn   = softmax(logits, axis=-1)
    agg    = pool @ attn.T                  [FEAT, L]
    o      = w_o @ agg                      [CH, L]
    out    = x + gamma * o

Sharding: data-parallel over (batch n, spatial half) -> 8 shards of
L=2048 pixel rows each; pool/weights replicated.

Per-core dataflow (all matmuls bf16 with fp32 PSUM accumulation):
    - layouts keep the pool axis p on SBUF partitions, so softmax
      normalization is deferred: we compute exp(logits) unnormalized
      (logits are bounded ~ +-25 here so no max-subtraction is needed),
      aggregate agg_unnorm = pool @ exp, and obtain the softmax
      denominator s[l] = sum_p exp via a cheap ones-vector matmul over a
      DVE-accumulated partial-sum tile.
    - The output projection is computed transposed, oT[l, c], so the
      1/s[l] normalization and the residual add are native per-partition
      DVE ops. gamma is folded into w_o on the host.

Host-side prep (cheap numpy): slices x per core, transposes weights/pool
to the layouts the matmuls want, casts to bf16, and transposes the
gathered per-core outputs back into [N, CH, H, W] float32.
"""

from contextlib import ExitStack

import numpy as np
import ml_dtypes

import concourse.tile as tile
from concourse import bacc, mybir
from concourse.bass_utils import run_bass_kernel_spmd

N, CH, H, W = 4, 512, 64, 64
HW = H * W                  # 4096
FEAT, POOL = 256, 8192
NCORES = 8
L = N * HW // NCORES        # 2048 pixel rows per core
NCHUNK = 4                  # l-chunks per core
LC = L // NCHUNK            # 512 = matmul moving size / PSUM bank
NPT = POOL // 128           # 64 pool partition-tiles
NSLOT = 8                   # DVE partial-sum accumulator slots
NSPL = 8                    # pool load-split count
PSPL = POOL // NSPL

BF16 = mybir.dt.bfloat16
F32 = mybir.dt.float32

LAST_RESULTS = None         # BassKernelResults of the most recent run
_NC_CACHE = None


def _emit(nc, tc, ctx, xb, xt, wt, wo, pl, plt, out):
    singles = ctx.enter_context(tc.tile_pool(name="singles", bufs=1))
    th_pool = ctx.enter_context(tc.tile_pool(name="th", bufs=2))
    exp_pool = ctx.enter_context(tc.tile_pool(name="expp", bufs=3))
    sacc_pool = ctx.enter_context(tc.tile_pool(name="sacc", bufs=1))
    agg_pool = ctx.enter_context(tc.tile_pool(name="aggsb", bufs=2))
    small_pool = ctx.enter_context(tc.tile_pool(name="small", bufs=2))
    outp = ctx.enter_context(tc.tile_pool(name="outp", bufs=3))
    ps_lg = ctx.enter_context(tc.tile_pool(name="pslg", bufs=2, space="PSUM"))
    ps_agg = ctx.enter_context(tc.tile_pool(name="psagg", bufs=1, space="PSUM"))
    ps_th = ctx.enter_context(tc.tile_pool(name="psth", bufs=1, space="PSUM"))
    ps_misc = ctx.enter_context(tc.tile_pool(name="psmisc", bufs=1, space="PSUM"))

    # resident inputs; spread across engine DMA queues, chunked finely and
    # emitted in consumption order so compute starts as soon as the first
    # slices land. The host pre-swizzles every input into the exact SBUF
    # tile layout, so each partition's read is one contiguous block and
    # the DMA engines run at line rate. xt (residual, needed only at
    # chunk tails) is loaded from inside the chunk loop so it doesn't
    # compete with the startup loads.
    wt_sb = singles.tile([128, CH // 128, FEAT], BF16)
    nc.sync.dma_start(out=wt_sb, in_=wt[:])
    xb_sbs = []
    for kc in range(NCHUNK):
        xb_i = singles.tile([128, CH // 128, LC], BF16, name=f"xb_{kc}")
        nc.sync.dma_start(out=xb_i, in_=xb[kc])
        xb_sbs.append(xb_i)
    pl_sbs, plt_sbs = [], []
    for i in range(NSPL):
        pl_i = singles.tile([128, FEAT // 128, PSPL], BF16, name=f"pl_{i}")
        nc.gpsimd.dma_start(out=pl_i, in_=pl[i])
        pl_sbs.append(pl_i)
        plt_i = singles.tile([128, NPT // NSPL, FEAT], BF16, name=f"plt_{i}")
        nc.gpsimd.dma_start(out=plt_i, in_=plt[i])
        plt_sbs.append(plt_i)
    wo_sb = singles.tile([128, FEAT // 128, CH], BF16)
    nc.scalar.dma_start(out=wo_sb, in_=wo[:])
    JC = LC // 128
    xt_sbs = [
        singles.tile([128, JC, CH], F32, name=f"xt_{kc}") for kc in range(NCHUNK)
    ]
    ones_sb = singles.tile([128, 1], F32)
    nc.vector.memset(ones_sb, 1.0)

    def pl_slice(ft, pt):
        i, r = divmod(pt, NPT // NSPL)
        return pl_sbs[i][:, ft, r * 128:(r + 1) * 128]

    def plt_slice(pt, m):
        i, r = divmod(pt, NPT // NSPL)
        return plt_sbs[i][:, r, m * 128:(m + 1) * 128]

    for kc in range(NCHUNK):
        # residual input for this chunk (needed only at the chunk tail)
        nc.scalar.dma_start(out=xt_sbs[kc], in_=xt[kc])

        # theta[f, l] for this l-chunk, bf16 in SBUF
        theta_sb = th_pool.tile([128, 2, LC], BF16)
        for m in range(2):
            ps_t = ps_th.tile([128, LC], F32, tag="th")
            for ct in range(CH // 128):
                nc.tensor.matmul(
                    ps_t,
                    lhsT=wt_sb[:, ct, m * 128:(m + 1) * 128],
                    rhs=xb_sbs[kc][:, ct, :],
                    start=(ct == 0),
                    stop=(ct == CH // 128 - 1),
                )
            nc.scalar.copy(theta_sb[:, m, :], ps_t)

        # streaming pool attention: logits -> exp -> aggregate.
        # s-partials use slot = pt // 8 so slots complete early and the
        # fold tree overlaps the tail of the loop.
        psum_agg = ps_agg.tile([128, 2, LC], F32)
        sacc = sacc_pool.tile([128, NSLOT, LC], BF16)
        sf = small_pool.tile([128, 7, LC], F32, tag="sfold")
        PERSLOT = NPT // NSLOT
        for pp in range(NPT // 2):
            lg = ps_lg.tile([128, 2, LC], F32)
            for k in range(2):
                pt = 2 * pp + k
                nc.tensor.matmul(
                    lg[:, k, :], lhsT=pl_slice(0, pt), rhs=theta_sb[:, 0, :],
                    start=True, stop=False,
                )
                nc.tensor.matmul(
                    lg[:, k, :], lhsT=pl_slice(1, pt), rhs=theta_sb[:, 1, :],
                    start=False, stop=True,
                )
            exp_t = exp_pool.tile([128, 2, LC], BF16)
            nc.scalar.activation(exp_t, lg, mybir.ActivationFunctionType.Exp)
            for k in range(2):
                pt = 2 * pp + k
                for m in range(2):
                    nc.tensor.matmul(
                        psum_agg[:, m, :],
                        lhsT=plt_slice(pt, m),
                        rhs=exp_t[:, k, :],
                        start=(pt == 0),
                        stop=(pt == NPT - 1),
                    )
                slot, r = divmod(pt, PERSLOT)
                if r == 0:
                    nc.vector.tensor_copy(sacc[:, slot, :], exp_t[:, k, :])
                else:
                    nc.vector.tensor_add(
                        sacc[:, slot, :], sacc[:, slot, :], exp_t[:, k, :]
                    )
                # progressive fold: slot q completes once pt == (q+1)*PERSLOT-1
                if r == PERSLOT - 1 and slot % 2 == 1:
                    q = slot // 2  # sf[q] = sacc[2q] + sacc[2q+1]
                    nc.vector.tensor_add(
                        sf[:, q, :], sacc[:, 2 * q, :], sacc[:, 2 * q + 1, :]
                    )
                    if q % 2 == 1:
                        nc.vector.tensor_add(
                            sf[:, 4 + q // 2, :], sf[:, q - 1, :], sf[:, q, :]
                        )

        # softmax denominator s[l]: final fold, then ones.T @ s
        nc.vector.tensor_add(sf[:, 6, :], sf[:, 4, :], sf[:, 5, :])
        ps_s = ps_misc.tile([1, LC], F32, tag="misc")
        nc.tensor.matmul(ps_s, lhsT=ones_sb, rhs=sf[:, 6, :], start=True, stop=True)
        s_row = small_pool.tile([1, LC], F32, tag="srow")
        nc.vector.tensor_copy(s_row, ps_s)

        # transpose s row -> per-partition column layout, reciprocal
        scol = small_pool.tile([128, LC // 128], F32, tag="scol")
        for j in range(LC // 128):
            nc.sync.dma_start(
                out=scol[:, j:j + 1], in_=s_row[0:1, j * 128:(j + 1) * 128]
            )
        rcol = small_pool.tile([128, LC // 128], F32, tag="rcol")
        nc.vector.reciprocal(rcol, scol)

        # agg psum -> sbuf bf16 (lhsT of the output projection)
        agg_sb = agg_pool.tile([128, 2, LC], BF16)
        for m in range(2):
            nc.scalar.copy(agg_sb[:, m, :], psum_agg[:, m, :])

        # oT[l, c] = agg.T @ (gamma*w_o.T); normalize by 1/s[l]; add x.T
        # (last chunk: alternate psum pools so the tail isn't serialized
        # on one bank; interior chunks overlap the next chunk's matmuls)
        for j in range(LC // 128):
            jl = kc * (LC // 128) + j
            if kc == NCHUNK - 1 and j % 2 == 1:
                ps_o = ps_th.tile([128, CH], F32, tag="th")
            else:
                ps_o = ps_misc.tile([128, CH], F32, tag="misc")
            for m in range(2):
                nc.tensor.matmul(
                    ps_o,
                    lhsT=agg_sb[:, m, j * 128:(j + 1) * 128],
                    rhs=wo_sb[:, m, :],
                    start=(m == 0),
                    stop=(m == 1),
                )
            out_sb = outp.tile([128, CH], F32)
            nc.vector.scalar_tensor_tensor(
                out=out_sb,
                in0=ps_o,
                scalar=rcol[:, j:j + 1],
                in1=xt_sbs[kc][:, j, :],
                op0=mybir.AluOpType.mult,
                op1=mybir.AluOpType.add,
            )
            nc.sync.dma_start(out=out[jl * 128:(jl + 1) * 128, :], in_=out_sb)


def _build_nc():
    nc = bacc.Bacc("TRN2", target_bir_lowering=False, debug=False)
    xb = nc.dram_tensor("xb", [NCHUNK, 128, CH // 128, LC], BF16, kind="ExternalInput")
    xt = nc.dram_tensor("xt", [NCHUNK, 128, LC // 128, CH], F32, kind="ExternalInput")
    wt = nc.dram_tensor("wt", [128, CH // 128, FEAT], BF16, kind="ExternalInput")
    wo = nc.dram_tensor("wo", [128, FEAT // 128, CH], BF16, kind="ExternalInput")
    pl = nc.dram_tensor("pl", [NSPL, 128, FEAT // 128, PSPL], BF16, kind="ExternalInput")
    plt = nc.dram_tensor("plt", [NSPL, 128, NPT // NSPL, FEAT], BF16, kind="ExternalInput")
    out = nc.dram_tensor("out", [L, CH], F32, kind="ExternalOutput")
    with tile.TileContext(nc) as tc, ExitStack() as ctx:
        _emit(nc, tc, ctx, xb[:], xt[:], wt[:], wo[:], pl[:], plt[:], out[:])
    nc.compile()
    return nc


def kernel(x, w_theta, w_o, concept_pool, gamma):
    global _NC_CACHE, LAST_RESULTS
    if _NC_CACHE is None:
        _NC_CACHE = _build_nc()
    nc = _NC_CACHE

    bf = ml_dtypes.bfloat16
    x = np.asarray(x, dtype=np.float32).reshape(N, CH, HW)
    # host-side swizzles into the exact SBUF tile layouts (see _emit)
    wt_h = np.ascontiguousarray(
        np.asarray(w_theta, np.float32).T.reshape(CH // 128, 128, FEAT)
        .transpose(1, 0, 2)
    ).astype(bf)
    wo_h = np.ascontiguousarray(
        (np.float32(gamma) * np.asarray(w_o, np.float32)).T
        .reshape(FEAT // 128, 128, CH).transpose(1, 0, 2)
    ).astype(bf)
    pool = np.asarray(concept_pool, np.float32)
    pl_h = np.ascontiguousarray(
        pool.reshape(FEAT // 128, 128, NSPL, PSPL).transpose(2, 1, 0, 3)
    ).astype(bf)
    plt_h = np.ascontiguousarray(
        pool.T.reshape(NSPL, NPT // NSPL, 128, FEAT).transpose(0, 2, 1, 3)
    ).astype(bf)

    in_maps = []
    for c in range(NCORES):
        n, half = divmod(c, NCORES // N)
        xc = x[n][:, half * L:(half + 1) * L]
        xb_h = np.ascontiguousarray(
            xc.reshape(CH // 128, 128, NCHUNK, LC).transpose(2, 1, 0, 3)
        ).astype(bf)
        xt_h = np.ascontiguousarray(
            xc.T.reshape(NCHUNK, LC // 128, 128, CH).transpose(0, 2, 1, 3)
        )
        in_maps.append({
            "xb": xb_h,
            "xt": xt_h,
            "wt": wt_h,
            "wo": wo_h,
            "pl": pl_h,
            "plt": plt_h,
        })

    LAST_RESULTS = run_bass_kernel_spmd(nc, in_maps, list(range(NCORES)))

    out = np.empty((N, CH, HW), dtype=np.float32)
    for c in range(NCORES):
        n, half = divmod(c, NCORES // N)
        out[n][:, half * L:(half + 1) * L] = LAST_RESULTS.results[c]["out"].T
    return out.reshape(N, CH, H, W)


# revision 20
# speedup vs baseline: 1.2443x; 1.0239x over previous
"""Concept-attention (vq_codebook) Trainium2 kernel.

Reference computation (per batch n, with L = H*W spatial positions):
    theta  = w_theta @ x_n                  [FEAT, L]
    logits = theta.T @ pool                 [L, POOL]
    attn   = softmax(logits, axis=-1)
    agg    = pool @ attn.T                  [FEAT, L]
    o      = w_o @ agg                      [CH, L]
    out    = x + gamma * o

Sharding: data-parallel over (batch n, spatial half) -> 8 shards of
L=2048 pixel rows each; pool/weights replicated.

Per-core dataflow (all matmuls bf16 with fp32 PSUM accumulation):
    - layouts keep the pool axis p on SBUF partitions, so softmax
      normalization is deferred: we compute exp(logits) unnormalized
      (logits are bounded ~ +-25 here so no max-subtraction is needed),
      aggregate agg_unnorm = pool @ exp, and obtain the softmax
      denominator s[l] = sum_p exp via a cheap ones-vector matmul over a
      DVE-accumulated partial-sum tile.
    - The output projection is computed transposed, oT[l, c], so the
      1/s[l] normalization and the residual add are native per-partition
      DVE ops. gamma is folded into w_o on the host.

Host-side prep (cheap numpy): slices x per core, transposes weights/pool
to the layouts the matmuls want, casts to bf16, and transposes the
gathered per-core outputs back into [N, CH, H, W] float32.
"""

from contextlib import ExitStack

import numpy as np
import ml_dtypes

import concourse.tile as tile
from concourse import bacc, mybir
from concourse.bass_utils import run_bass_kernel_spmd

N, CH, H, W = 4, 512, 64, 64
HW = H * W                  # 4096
FEAT, POOL = 256, 8192
NCORES = 8
L = N * HW // NCORES        # 2048 pixel rows per core
NCHUNK = 4                  # l-chunks per core
LC = L // NCHUNK            # 512 = matmul moving size / PSUM bank
NPT = POOL // 128           # 64 pool partition-tiles
NSLOT = 8                   # DVE partial-sum accumulator slots
NSPL = 8                    # pool load-split count
PSPL = POOL // NSPL

BF16 = mybir.dt.bfloat16
F32 = mybir.dt.float32

LAST_RESULTS = None         # BassKernelResults of the most recent run
_NC_CACHE = None


def _emit(nc, tc, ctx, xb, xt, wt, wo, pl, plt, out):
    singles = ctx.enter_context(tc.tile_pool(name="singles", bufs=1))
    th_pool = ctx.enter_context(tc.tile_pool(name="th", bufs=2))
    exp_pool = ctx.enter_context(tc.tile_pool(name="expp", bufs=3))
    sacc_pool = ctx.enter_context(tc.tile_pool(name="sacc", bufs=1))
    agg_pool = ctx.enter_context(tc.tile_pool(name="aggsb", bufs=2))
    small_pool = ctx.enter_context(tc.tile_pool(name="small", bufs=2))
    outp = ctx.enter_context(tc.tile_pool(name="outp", bufs=3))
    ps_lg = ctx.enter_context(tc.tile_pool(name="pslg", bufs=2, space="PSUM"))
    ps_agg = ctx.enter_context(tc.tile_pool(name="psagg", bufs=1, space="PSUM"))
    ps_th = ctx.enter_context(tc.tile_pool(name="psth", bufs=1, space="PSUM"))
    ps_misc = ctx.enter_context(tc.tile_pool(name="psmisc", bufs=1, space="PSUM"))

    # resident inputs; spread across engine DMA queues, chunked finely and
    # emitted in consumption order so compute starts as soon as the first
    # slices land. The host pre-swizzles every input into the exact SBUF
    # tile layout, so each partition's read is one contiguous block and
    # the DMA engines run at line rate. xt (residual, needed only at
    # chunk tails) is loaded from inside the chunk loop so it doesn't
    # compete with the startup loads.
    wt_sb = singles.tile([128, CH // 128, FEAT], BF16)
    nc.sync.dma_start(out=wt_sb, in_=wt[:])
    xb_sbs = []
    xb_0 = singles.tile([128, CH // 128, LC], BF16, name="xb_0")
    nc.sync.dma_start(out=xb_0, in_=xb[0])
    xb_sbs.append(xb_0)
    pl_sbs, plt_sbs = [], []
    for i in range(NSPL):
        if 1 <= i <= NCHUNK - 1:
            xb_i = singles.tile([128, CH // 128, LC], BF16, name=f"xb_{i}")
            nc.sync.dma_start(out=xb_i, in_=xb[i])
            xb_sbs.append(xb_i)
        pl_i = singles.tile([128, FEAT // 128, PSPL], BF16, name=f"pl_{i}")
        nc.sync.dma_start(out=pl_i, in_=pl[i])
        pl_sbs.append(pl_i)
        plt_i = singles.tile([128, NPT // NSPL, FEAT], BF16, name=f"plt_{i}")
        nc.sync.dma_start(out=plt_i, in_=plt[i])
        plt_sbs.append(plt_i)
    wo_sb = singles.tile([128, FEAT // 128, CH], BF16)
    nc.sync.dma_start(out=wo_sb, in_=wo[:])
    JC = LC // 128
    xt_sbs = [
        singles.tile([128, JC, CH], F32, name=f"xt_{kc}") for kc in range(NCHUNK)
    ]
    ones_sb = singles.tile([128, 1], F32)
    nc.vector.memset(ones_sb, 1.0)

    def pl_slice(ft, pt):
        i, r = divmod(pt, NPT // NSPL)
        return pl_sbs[i][:, ft, r * 128:(r + 1) * 128]

    def plt_slice(pt, m):
        i, r = divmod(pt, NPT // NSPL)
        return plt_sbs[i][:, r, m * 128:(m + 1) * 128]

    for kc in range(NCHUNK):
        # residual input for this chunk (needed only at the chunk tail)
        nc.sync.dma_start(out=xt_sbs[kc], in_=xt[kc])

        # theta[f, l] for this l-chunk, bf16 in SBUF
        theta_sb = th_pool.tile([128, 2, LC], BF16)
        for m in range(2):
            ps_t = ps_th.tile([128, LC], F32, tag="th")
            for ct in range(CH // 128):
                nc.tensor.matmul(
                    ps_t,
                    lhsT=wt_sb[:, ct, m * 128:(m + 1) * 128],
                    rhs=xb_sbs[kc][:, ct, :],
                    start=(ct == 0),
                    stop=(ct == CH // 128 - 1),
                )
            nc.scalar.copy(theta_sb[:, m, :], ps_t)

        # streaming pool attention: logits -> exp -> aggregate.
        # s-partials use slot = pt // 8 so slots complete early and the
        # fold tree overlaps the tail of the loop.
        psum_agg = ps_agg.tile([128, 2, LC], F32)
        sacc = sacc_pool.tile([128, NSLOT, LC], BF16)
        sf = small_pool.tile([128, 7, LC], F32, tag="sfold")
        PERSLOT = NPT // NSLOT
        for pp in range(NPT // 2):
            lg = ps_lg.tile([128, 2, LC], F32)
            for k in range(2):
                pt = 2 * pp + k
                nc.tensor.matmul(
                    lg[:, k, :], lhsT=pl_slice(0, pt), rhs=theta_sb[:, 0, :],
                    start=True, stop=False,
                )
                nc.tensor.matmul(
                    lg[:, k, :], lhsT=pl_slice(1, pt), rhs=theta_sb[:, 1, :],
                    start=False, stop=True,
                )
            exp_t = exp_pool.tile([128, 2, LC], BF16)
            nc.scalar.activation(exp_t, lg, mybir.ActivationFunctionType.Exp)
            for k in range(2):
                pt = 2 * pp + k
                for m in range(2):
                    nc.tensor.matmul(
                        psum_agg[:, m, :],
                        lhsT=plt_slice(pt, m),
                        rhs=exp_t[:, k, :],
                        start=(pt == 0),
                        stop=(pt == NPT - 1),
                    )
                slot, r = divmod(pt, PERSLOT)
                if r == 0:
                    nc.vector.tensor_copy(sacc[:, slot, :], exp_t[:, k, :])
                else:
                    nc.vector.tensor_add(
                        sacc[:, slot, :], sacc[:, slot, :], exp_t[:, k, :]
                    )
                # progressive fold: slot q completes once pt == (q+1)*PERSLOT-1
                if r == PERSLOT - 1 and slot % 2 == 1:
                    q = slot // 2  # sf[q] = sacc[2q] + sacc[2q+1]
                    nc.vector.tensor_add(
                        sf[:, q, :], sacc[:, 2 * q, :], sacc[:, 2 * q + 1, :]
                    )
                    if q % 2 == 1:
                        nc.vector.tensor_add(
                            sf[:, 4 + q // 2, :], sf[:, q - 1, :], sf[:, q, :]
                        )

        # softmax denominator s[l]: final fold, then ones.T @ s
        nc.vector.tensor_add(sf[:, 6, :], sf[:, 4, :], sf[:, 5, :])
        ps_s = ps_misc.tile([1, LC], F32, tag="misc")
        nc.tensor.matmul(ps_s, lhsT=ones_sb, rhs=sf[:, 6, :], start=True, stop=True)
        s_row = small_pool.tile([1, LC], F32, tag="srow")
        nc.vector.tensor_copy(s_row, ps_s)

        # transpose s row -> per-partition column layout, reciprocal
        scol = small_pool.tile([128, LC // 128], F32, tag="scol")
        for j in range(LC // 128):
            nc.sync.dma_start(
                out=scol[:, j:j + 1], in_=s_row[0:1, j * 128:(j + 1) * 128]
            )
        rcol = small_pool.tile([128, LC // 128], F32, tag="rcol")
        nc.vector.reciprocal(rcol, scol)

        # agg psum -> sbuf bf16 (lhsT of the output projection)
        agg_sb = agg_pool.tile([128, 2, LC], BF16)
        for m in range(2):
            nc.scalar.copy(agg_sb[:, m, :], psum_agg[:, m, :])

        # oT[l, c] = agg.T @ (gamma*w_o.T); normalize by 1/s[l]; add x.T
        # (last chunk: alternate psum pools so the tail isn't serialized
        # on one bank; interior chunks overlap the next chunk's matmuls)
        for j in range(LC // 128):
            jl = kc * (LC // 128) + j
            if kc == NCHUNK - 1 and j % 2 == 1:
                ps_o = ps_th.tile([128, CH], F32, tag="th")
            else:
                ps_o = ps_misc.tile([128, CH], F32, tag="misc")
            for m in range(2):
                nc.tensor.matmul(
                    ps_o,
                    lhsT=agg_sb[:, m, j * 128:(j + 1) * 128],
                    rhs=wo_sb[:, m, :],
                    start=(m == 0),
                    stop=(m == 1),
                )
            out_sb = outp.tile([128, CH], F32)
            nc.vector.scalar_tensor_tensor(
                out=out_sb,
                in0=ps_o,
                scalar=rcol[:, j:j + 1],
                in1=xt_sbs[kc][:, j, :],
                op0=mybir.AluOpType.mult,
                op1=mybir.AluOpType.add,
            )
            nc.sync.dma_start(out=out[jl * 128:(jl + 1) * 128, :], in_=out_sb)


def _build_nc():
    nc = bacc.Bacc("TRN2", target_bir_lowering=False, debug=False)
    xb = nc.dram_tensor("xb", [NCHUNK, 128, CH // 128, LC], BF16, kind="ExternalInput")
    xt = nc.dram_tensor("xt", [NCHUNK, 128, LC // 128, CH], F32, kind="ExternalInput")
    wt = nc.dram_tensor("wt", [128, CH // 128, FEAT], BF16, kind="ExternalInput")
    wo = nc.dram_tensor("wo", [128, FEAT // 128, CH], BF16, kind="ExternalInput")
    pl = nc.dram_tensor("pl", [NSPL, 128, FEAT // 128, PSPL], BF16, kind="ExternalInput")
    plt = nc.dram_tensor("plt", [NSPL, 128, NPT // NSPL, FEAT], BF16, kind="ExternalInput")
    out = nc.dram_tensor("out", [L, CH], F32, kind="ExternalOutput")
    with tile.TileContext(nc) as tc, ExitStack() as ctx:
        _emit(nc, tc, ctx, xb[:], xt[:], wt[:], wo[:], pl[:], plt[:], out[:])
    nc.compile()
    return nc


def kernel(x, w_theta, w_o, concept_pool, gamma):
    global _NC_CACHE, LAST_RESULTS
    if _NC_CACHE is None:
        _NC_CACHE = _build_nc()
    nc = _NC_CACHE

    bf = ml_dtypes.bfloat16
    x = np.asarray(x, dtype=np.float32).reshape(N, CH, HW)
    # host-side swizzles into the exact SBUF tile layouts (see _emit)
    wt_h = np.ascontiguousarray(
        np.asarray(w_theta, np.float32).T.reshape(CH // 128, 128, FEAT)
        .transpose(1, 0, 2)
    ).astype(bf)
    wo_h = np.ascontiguousarray(
        (np.float32(gamma) * np.asarray(w_o, np.float32)).T
        .reshape(FEAT // 128, 128, CH).transpose(1, 0, 2)
    ).astype(bf)
    pool = np.asarray(concept_pool, np.float32)
    pl_h = np.ascontiguousarray(
        pool.reshape(FEAT // 128, 128, NSPL, PSPL).transpose(2, 1, 0, 3)
    ).astype(bf)
    plt_h = np.ascontiguousarray(
        pool.T.reshape(NSPL, NPT // NSPL, 128, FEAT).transpose(0, 2, 1, 3)
    ).astype(bf)

    in_maps = []
    for c in range(NCORES):
        n, half = divmod(c, NCORES // N)
        xc = x[n][:, half * L:(half + 1) * L]
        xb_h = np.ascontiguousarray(
            xc.reshape(CH // 128, 128, NCHUNK, LC).transpose(2, 1, 0, 3)
        ).astype(bf)
        xt_h = np.ascontiguousarray(
            xc.T.reshape(NCHUNK, LC // 128, 128, CH).transpose(0, 2, 1, 3)
        )
        in_maps.append({
            "xb": xb_h,
            "xt": xt_h,
            "wt": wt_h,
            "wo": wo_h,
            "pl": pl_h,
            "plt": plt_h,
        })

    LAST_RESULTS = run_bass_kernel_spmd(nc, in_maps, list(range(NCORES)))

    out = np.empty((N, CH, HW), dtype=np.float32)
    for c in range(NCORES):
        n, half = divmod(c, NCORES // N)
        out[n][:, half * L:(half + 1) * L] = LAST_RESULTS.results[c]["out"].T
    return out.reshape(N, CH, H, W)
